# revision 5
# baseline (speedup 1.0000x reference)
"""Trainium2 Bass kernel for nn_Attention (linear attention, no softmax).

Key identity: without softmax, (Q K^T) V = Q (K^T V), so the whole block
collapses to per-batch [C,C] matrices:
    S    = xs^T xs                     [C,C]   (xs = [L,C] tokens)
    At_h = Wk_h^T Wq_h  (= A_h^T)      [C,C]   batch-independent -> host-folded
    B_h  = Wv_h^T Wo_h^T               [C,C]   batch-independent -> host-folded
    Tt_h = S At_h   (= (A_h S)^T)      [C,C]
    G    = sum_h Tt_h^T B_h            [C,C]
    out  = (G^T X) + bias              [C,L]   (X = xs^T, the native x layout)

Sharding: data-parallel over batch, 2 batches per core across 8 cores.
bf16 on the wire and in the PE (f32 PSUM accumulate).

Perf notes (from traces):
- PE HAM clock gate: 1.2 GHz cold -> 2.4 GHz after ~3.4us of sustained PE
  activity. Warmup matmuls run while the input DMA lands so the real
  stream runs warm.
- DMA engines take ~1.1us to pick up the first transfer after idle; a
  1-descriptor dummy load on each ring starts that clock early.
- Aggregate DMA bandwidth is ~400 GB/s shared by both HWDGE rings. Every
  input tensor is split in half across BOTH rings, ordered by consumption
  (xs0, xs1, at, b, x0, x1, bias), so each stage's data lands just ahead
  of the PE stream (the v1 kernel stalled 1.1us waiting for `at`).
- Output chunks are written as partition-halves [64, L] on both rings:
  2KB descriptors (full DMA efficiency), half the descriptor-gen time per
  engine, and both halves fly concurrently.
- Per (b, m) chunk the two bias-adds run on scalar (activation) and
  vector (tensor_scalar_add) in parallel.
"""

import numpy as np

P = 128
B_FULL, C, W, H = 16, 256, 32, 32
L = W * H  # 1024
NH = 4
NCORES = 8
BPC = B_FULL // NCORES  # batches per core = 2
CT = C // P   # 2 c-tiles
LT = L // P   # 8 L-tiles
HLT = LT // 2  # lt-groups per xs half

_CACHE = {}


def _np_bf16():
    import ml_dtypes
    return ml_dtypes.bfloat16


def _build_program():
    import concourse.bacc as bacc
    import concourse.mybir as mybir
    import concourse.tile as tile

    f32 = mybir.dt.float32
    mmdt = mybir.dt.bfloat16

    nc = bacc.Bacc("TRN2", target_bir_lowering=False, debug=False)

    # All inputs host-packed to [128, free] partition-major layouts.
    xs_d = nc.dram_tensor("xs", [BPC, P, LT * C], mmdt, kind="ExternalInput").ap()
    at_d = nc.dram_tensor("at", [P, CT * NH * C], mmdt, kind="ExternalInput").ap()
    b_d = nc.dram_tensor("b", [P, CT * NH * C], mmdt, kind="ExternalInput").ap()
    x2d_d = nc.dram_tensor("x2d", [BPC, P, CT * L], mmdt, kind="ExternalInput").ap()
    wob_d = nc.dram_tensor("wob", [P, CT], f32, kind="ExternalInput").ap()
    out_d = nc.dram_tensor("out", [P, BPC * CT * L], mmdt, kind="ExternalOutput").ap()

    with tile.TileContext(nc) as tc:
        from contextlib import ExitStack

        with ExitStack() as ctx:
            const = ctx.enter_context(tc.tile_pool(name="const", bufs=1))
            work = ctx.enter_context(tc.tile_pool(name="work", bufs=1))
            zpool = ctx.enter_context(tc.tile_pool(name="zout", bufs=4))
            psum = ctx.enter_context(tc.tile_pool(name="psum", bufs=8, space="PSUM"))

            def mm(ps_ap, lhsT_ap, rhs_ap, start, stop):
                nc.tensor.matmul(ps_ap, lhsT_ap, rhs_ap, start=start, stop=stop)

            # ---- SBUF tiles: every big input split in ring-halves
            wake_sb = [const.tile([1, 64], mybir.dt.int32, tag=f"wk{r}",
                                  name=f"wake{r}") for r in range(2)]
            # xs halves: [b][h] h=0 -> lt 0..3, h=1 -> lt 4..7
            xs_sb = [[work.tile([P, HLT * C], mmdt, tag=f"xs{b}{h}",
                                name=f"xs_sb{b}{h}") for h in range(2)]
                     for b in range(BPC)]
            # at/b halves: kt-split
            at_sb = [const.tile([P, NH * C], mmdt, tag=f"at{k}", name=f"at{k}")
                     for k in range(CT)]
            b_sb = [const.tile([P, NH * C], mmdt, tag=f"b{k}", name=f"bm{k}")
                    for k in range(CT)]
            # x2d halves: kt-split
            x_sb = [[work.tile([P, L], mmdt, tag=f"x{b}{k}", name=f"x_sb{b}{k}")
                     for k in range(CT)] for b in range(BPC)]
            bias_sb = const.tile([P, CT], f32, tag="bias")
            wu_sb = const.tile([P, P], mmdt, tag="wu")
            wu_ps = psum.tile([P, 512], f32, tag="ps", name="wu_ps")

            # PE warmup: releases the HAM clock gate (~3.4us of activity)
            # while the input DMA lands. memset on gpsimd (its queue opens
            # right after the preamble barrier).
            nc.gpsimd.memset(wu_sb[:], 0.0)
            for i in range(21):
                nc.tensor.matmul(wu_ps[:, :P], wu_sb[:], wu_sb[:],
                                 start=True, stop=True)

            # ---- input DMA: dummy wake first, then halves of each tensor
            # on both rings in consumption order.
            nc.sync.dma_start(wake_sb[0][:], nc.dummy.ap())
            nc.scalar.dma_start(wake_sb[1][:], nc.dummy.ap())
            half = HLT * C
            for b in range(BPC):
                nc.sync.dma_start(xs_sb[b][0][:], xs_d[b][:, :half])
                nc.scalar.dma_start(xs_sb[b][1][:], xs_d[b][:, half:])
            nc.sync.dma_start(at_sb[0][:], at_d[:, :NH * C])
            nc.scalar.dma_start(at_sb[1][:], at_d[:, NH * C:])
            nc.sync.dma_start(b_sb[0][:], b_d[:, :NH * C])
            nc.scalar.dma_start(b_sb[1][:], b_d[:, NH * C:])
            for b in range(BPC):
                nc.sync.dma_start(x_sb[b][0][:], x2d_d[b][:, :L])
                nc.scalar.dma_start(x_sb[b][1][:], x2d_d[b][:, L:])
            nc.sync.dma_start(bias_sb[:], wob_d[:])

            def copy_halves(dst_lo, src_lo, dst_hi, src_hi):
                nc.any.tensor_copy(dst_lo, src_lo)
                nc.any.tensor_copy(dst_hi, src_hi)

            # ---- S = xs^T xs per batch (symmetric)
            s_sb = [work.tile([P, CT * C], mmdt, tag=f"s{b}", name=f"s_sb{b}") for b in range(BPC)]

            def s_stage(b):
                ps = psum.tile([P, 512], f32, tag="ps")
                for m in range(CT):
                    for lt in range(LT):
                        xh = xs_sb[b][lt // HLT]
                        lo = (lt % HLT) * C
                        mm(ps[:, m * C:(m + 1) * C],
                           xh[:, lo + m * P: lo + m * P + P],
                           xh[:, lo:lo + C],
                           lt == 0, lt == LT - 1)
                copy_halves(s_sb[b][:, :C], ps[:, :C], s_sb[b][:, C:], ps[:, C:])

            # ---- Tt_h = S At_h ; layout [P, m*NH*C] like at layout
            tt_sb = [work.tile([P, CT * NH * C], mmdt, tag=f"tt{b}", name=f"tt_sb{b}") for b in range(BPC)]

            def tt_stage(b):
                for m in range(CT):
                    pss = [psum.tile([P, 512], f32, tag="ps", name=f"ps_tt{m}_{i}") for i in range(NH // 2)]
                    for kt in range(CT):
                        for hp in range(NH // 2):  # consecutive mms share lhsT
                            mm(pss[hp][:],
                               s_sb[b][:, kt * C + m * P: kt * C + m * P + P],
                               at_sb[kt][:, (hp * 2) * C:(hp * 2 + 2) * C],
                               kt == 0, kt == CT - 1)
                    o = (m * NH) * C
                    copy_halves(tt_sb[b][:, o:o + 512], pss[0][:],
                                tt_sb[b][:, o + 512:o + 1024], pss[1][:])

            # ---- G = sum_h Tt_h^T B_h
            g_sb = [work.tile([P, CT * C], mmdt, tag=f"g{b}", name=f"g_sb{b}") for b in range(BPC)]

            def g_stage(b):
                ps = psum.tile([P, 512], f32, tag="ps")
                for m in range(CT):
                    i, n_acc = 0, NH * CT
                    for h in range(NH):
                        for kt in range(CT):
                            mm(ps[:, m * C:(m + 1) * C],
                               tt_sb[b][:, (kt * NH + h) * C + m * P:(kt * NH + h) * C + m * P + P],
                               b_sb[kt][:, h * C:(h + 1) * C],
                               i == 0, i == n_acc - 1)
                            i += 1
                copy_halves(g_sb[b][:, :C], ps[:, :C], g_sb[b][:, C:], ps[:, C:])

            # ---- out = G^T X + bias
            # bias-add halves on scalar+vector in parallel, then the [P, L]
            # chunk is DMA'd as two partition-halves on both rings (2KB
            # descriptors, half the gen latency, concurrent wire time).
            def z_stage(b):
                for m in range(CT):
                    pss = [psum.tile([P, 512], f32, tag="ps", name=f"ps_z{m}_{i}") for i in range(2)]
                    for kt in range(CT):
                        for nt in range(2):  # consecutive mms share lhsT
                            mm(pss[nt][:],
                               g_sb[b][:, kt * C + m * P: kt * C + m * P + P],
                               x_sb[b][kt][:, nt * 512:(nt + 1) * 512],
                               kt == 0, kt == CT - 1)
                    zb = zpool.tile([P, L], mmdt, tag="z")
                    bias_ap = bias_sb[:, m:m + 1]
                    nc.scalar.activation(
                        zb[:, :512], pss[0][:],
                        mybir.ActivationFunctionType.Identity, bias=bias_ap)
                    nc.vector.tensor_scalar_add(zb[:, 512:], pss[1][:], bias_ap)
                    base = (b * CT + m) * L
                    nc.scalar.dma_start(out_d[:P // 2, base:base + L], zb[:P // 2, :])
                    nc.sync.dma_start(out_d[P // 2:, base:base + L], zb[P // 2:, :])

            # ---- schedule: interleave batches to keep PE gap-free
            s_stage(0)
            s_stage(1)
            tt_stage(0)
            tt_stage(1)
            g_stage(0)
            g_stage(1)
            z_stage(0)
            z_stage(1)

    nc.compile()
    return nc


def _get_program():
    if "nc" not in _CACHE:
        _CACHE["nc"] = _build_program()
    return _CACHE["nc"]


def _pack_rows(a, tiles):
    """[tiles*P, F] row-major -> [P, tiles*F] partition-major."""
    tP, F = a.shape
    assert tP == tiles * P
    return np.ascontiguousarray(
        a.reshape(tiles, P, F).transpose(1, 0, 2).reshape(P, tiles * F))


def _pack_w(Wt, ndt):
    """[NH, C, C] -> [P, CT*NH*C]: dst[p, (m*NH+h)*C+j] = Wt[h, m*P+p, j]."""
    a = np.asarray(Wt, np.float32).reshape(NH, CT, P, C)
    return np.ascontiguousarray(
        a.transpose(2, 1, 0, 3).reshape(P, CT * NH * C)).astype(ndt)


def _prep_inputs(x, Wq, Wk, Wv, Wo_w, Wo_b):
    ndt = _np_bf16()
    x = np.asarray(x, dtype=np.float32)
    X = x.reshape(B_FULL, C, L)                                    # [b, C, L]
    XS = X.transpose(0, 2, 1)                                      # [b, L, C]
    Wq = np.asarray(Wq, np.float32)
    Wk = np.asarray(Wk, np.float32)
    Wv = np.asarray(Wv, np.float32)
    WoT = np.ascontiguousarray(np.asarray(Wo_w, np.float32).T).reshape(NH, C, C)

    # fold the batch-independent weight products on host
    At = np.einsum('hdc,hde->hce', Wk, Wq)   # At_h = Wk_h^T Wq_h
    Bm = np.einsum('hdc,hde->hce', Wv, WoT)  # B_h  = Wv_h^T WoT_h

    common = {
        "at": _pack_w(At, ndt), "b": _pack_w(Bm, ndt),
        "wob": np.ascontiguousarray(
            np.asarray(Wo_b, np.float32).reshape(CT, P).T),
    }
    in_maps = []
    for i in range(NCORES):
        bs = slice(i * BPC, (i + 1) * BPC)
        x2d_p = np.stack([_pack_rows(Xb, CT) for Xb in X[bs]]).astype(ndt)
        xs_p = np.stack([_pack_rows(Sb, LT) for Sb in XS[bs]]).astype(ndt)
        in_maps.append({"x2d": x2d_p, "xs": xs_p, **common})
    return in_maps


def _unpack_out(res_list):
    """per-core [P, BPC*CT*L] -> [B_FULL, C, W, H]"""
    out = np.empty((B_FULL, C, L), dtype=np.float32)
    for i in range(NCORES):
        o = np.asarray(res_list[i]["out"], dtype=np.float32).reshape(P, BPC, CT, L)
        for b in range(BPC):
            out[i * BPC + b] = o[:, b].transpose(1, 0, 2).reshape(C, L)
    return out.reshape(B_FULL, C, W, H)


def run_sharded(inputs, trace=False, trace_cores=None):
    """Run the SPMD kernel; returns (full_output, BassKernelResults)."""
    from concourse.bass_utils import run_bass_kernel_spmd

    in_maps = _prep_inputs(**inputs)
    nc = _get_program()
    res = run_bass_kernel_spmd(
        nc, in_maps, core_ids=list(range(NCORES)),
        trace=trace, trace_cores=trace_cores,
    )
    return _unpack_out(res.results), res


def kernel(x, Wq, Wk, Wv, Wo_w, Wo_b):
    out, _ = run_sharded(
        {"x": x, "Wq": Wq, "Wk": Wk, "Wv": Wv, "Wo_w": Wo_w, "Wo_b": Wo_b}
    )
    return out
